# revision 19
# baseline (speedup 1.0000x reference)
"""2-rows-per-partition variant: each SBUF tile [128, 4096] holds 256
output rows (partition p carries rows i0+2p and i0+2p+1 back to back),
so each DMA moves 2 MiB with 16-KB-contiguous runs per partition.  If
the DGE emits one descriptor per run, descriptor overhead halves
(modeled 26.0 -> ~27.7 GB/s/engine, stream 80 -> ~75us).

W image: W2[p, c] = V(c - 2p - (S-1)), c in [254, 4095), cc = c - 254.
Block b (rows i0=256b .. i0+255), half r: TB[p, 2048r + j] =
V(j - i0 - 2p - r) = W2 at cc0 + j, cc0 = 1793 - 256b - r.
"""

import numpy as np

H = 16
S = 2048
P = 128
N_CORES = 8
H_LOC = H // N_CORES
NB = 8          # 256-row blocks per head
BR = 2 * S      # tile free width (two 2048 halves)

W2_LO = 1792    # lo chunk: cc in [0, 1792)
W2_HI = 2049    # hi chunk: cc in [1792, 3841)

_NC = None


def _build(nbuf=7):
    import concourse.bacc as bacc
    import concourse.mybir as mybir
    from concourse.tile import TileContext

    f32 = mybir.dt.float32
    nc = bacc.Bacc("TRN2", target_bir_lowering=False, debug=False)

    alpha_d = nc.dram_tensor("alpha", [H_LOC], f32, kind="ExternalInput").ap()
    beta_d = nc.dram_tensor("beta", [H_LOC], f32, kind="ExternalInput").ap()
    gamma_d = nc.dram_tensor("gamma", [H_LOC], f32, kind="ExternalInput").ap()
    out_d = nc.dram_tensor("out", [H_LOC, S, S], f32, kind="ExternalOutput").ap()

    with TileContext(nc) as tc:
        rings = [nc.sync, nc.scalar]

        with (
            tc.tile_pool(name="coef", bufs=1) as cpool,
            tc.tile_pool(name="kpool", bufs=1) as kpool,
            tc.tile_pool(name="wpool", bufs=1) as wpool,
            tc.tile_pool(name="t2pool", bufs=2) as t2pool,
            tc.tile_pool(name="tpool", bufs=nbuf) as tpool,
        ):
            G2 = cpool.tile([P, H_LOC], f32)
            nc.sync.dma_start(out=G2[:], in_=gamma_d.partition_broadcast(P))
            B2 = cpool.tile([P, H_LOC], f32)
            nc.scalar.dma_start(out=B2[:], in_=beta_d.partition_broadcast(P))
            A2 = cpool.tile([P, H_LOC], f32)
            nc.sync.dma_start(out=A2[:], in_=alpha_d.partition_broadcast(P))
            NB2 = cpool.tile([P, H_LOC], f32)
            nc.vector.tensor_scalar_mul(NB2[:], B2[:], -1.0)

            # K2hi[p, cc'] = cc' - 2p - 1  (cc = cc' + 1792)
            K2hi = kpool.tile([P, W2_HI], f32, tag="K2hi")
            nc.gpsimd.iota(
                K2hi[:],
                pattern=[[1, W2_HI]],
                base=-1,
                channel_multiplier=-2,
                allow_small_or_imprecise_dtypes=True,
            )
            # IB2[p, 2b+r] = 2p + 256b + r
            IB2 = cpool.tile([P, 2 * NB], f32, tag="IB2")
            nc.gpsimd.iota(
                IB2[:],
                pattern=[[256, NB], [1, 2]],
                base=0,
                channel_multiplier=2,
                allow_small_or_imprecise_dtypes=True,
            )
            # K2lo[p, cc] = cc - 2p - 1793
            K2lo = kpool.tile([P, W2_LO], f32, tag="K2lo")
            nc.gpsimd.iota(
                K2lo[:],
                pattern=[[1, W2_LO]],
                base=-1793,
                channel_multiplier=-2,
                allow_small_or_imprecise_dtypes=True,
            )
            Rs = [None, None]

            def w_chunk(Kc, w, h, Wout):
                T2 = t2pool.tile([P, W2_HI], f32, tag="T2")
                if h == 1:
                    nc.scalar.activation(
                        out=T2[:, :w],
                        in_=Kc[:, :w],
                        func=mybir.ActivationFunctionType.Relu,
                        scale=G2[:, h : h + 1],
                    )
                else:
                    nc.vector.tensor_scalar(
                        out=T2[:, :w],
                        in0=Kc[:, :w],
                        scalar1=G2[:, h : h + 1],
                        scalar2=0.0,
                        op0=mybir.AluOpType.mult,
                        op1=mybir.AluOpType.max,
                    )
                nc.vector.scalar_tensor_tensor(
                    out=Wout[:],
                    in0=Kc[:, :w],
                    scalar=NB2[:, h : h + 1],
                    in1=T2[:, :w],
                    op0=mybir.AluOpType.mult,
                    op1=mybir.AluOpType.max,
                )

            Whi = [wpool.tile([P, W2_HI], f32, tag=f"Whi{h}", name=f"Whi{h}") for h in range(H_LOC)]
            Wlo = [wpool.tile([P, W2_LO], f32, tag=f"Wlo{h}", name=f"Wlo{h}") for h in range(H_LOC)]

            def cpy(h, out, in_):
                if h == 0:
                    nc.vector.tensor_copy(out=out, in_=in_)
                else:
                    nc.scalar.copy(out=out, in_=in_)

            def emit_tile(h, b):
                if Rs[h] is None:
                    Rh = cpool.tile([P, 2 * NB], f32, tag=f"R{h}", name=f"R{h}")
                    nc.vector.tensor_scalar_mul(Rh[:], IB2[:], A2[:, h : h + 1])
                    Rs[h] = Rh
                T = tpool.tile([P, BR], f32, tag="T")
                for r in range(2):
                    cc0 = 1793 - 256 * b - r
                    d0 = 2048 * r
                    if b == 0:
                        # entirely in the hi chunk: hi-local [cc0-1792, +2048)
                        hl = cc0 - W2_LO
                        cpy(h, T[:, d0 : d0 + S], Whi[h][:, hl : hl + S])
                    else:
                        wlo = W2_LO - cc0
                        cpy(h, T[:, d0 : d0 + wlo], Wlo[h][:, cc0:W2_LO])
                        cpy(h, T[:, d0 + wlo : d0 + S], Whi[h][:, 0 : S - wlo])
                    # column-0 patch for rows i0+2p+r
                    cpy(h, T[:, d0 : d0 + 1], Rs[h][:, 2 * b + r : 2 * b + r + 1])
                if b == 0:
                    # row 0 (p=0, r=0): alpha*j; K2hi[0, cc'] = cc'-1, j = cc'-1
                    if h == 0:
                        nc.vector.tensor_scalar_mul(
                            T[0:1, 0:S], K2hi[0:1, 1 : S + 1], A2[0:1, h : h + 1]
                        )
                    else:
                        nc.scalar.mul(
                            T[0:1, 0:S], K2hi[0:1, 1 : S + 1], A2[0:1, h : h + 1]
                        )
                dst = out_d[h, 256 * b : 256 * (b + 1), :].rearrange(
                    "(p q) j -> p (q j)", p=P
                )
                rings[h].dma_start(out=dst, in_=T[:])

            w_chunk(K2hi, W2_HI, 0, Whi[0])
            emit_tile(0, 0)
            w_chunk(K2hi, W2_HI, 1, Whi[1])
            emit_tile(1, 0)
            w_chunk(K2lo, W2_LO, 0, Wlo[0])
            emit_tile(0, 1)
            w_chunk(K2lo, W2_LO, 1, Wlo[1])
            emit_tile(1, 1)
            for b in range(2, NB):
                for h in range(H_LOC):
                    emit_tile(h, b)

    nc.compile()
    return nc


def _run(alpha, beta, gamma, **spmd_kwargs):
    global _NC
    if _NC is None:
        _NC = _build()
    from concourse import bass_utils

    alpha = np.ascontiguousarray(alpha, dtype=np.float32)
    beta = np.ascontiguousarray(beta, dtype=np.float32)
    gamma = np.ascontiguousarray(gamma, dtype=np.float32)
    in_maps = [
        {
            "alpha": alpha[c * H_LOC : (c + 1) * H_LOC],
            "beta": beta[c * H_LOC : (c + 1) * H_LOC],
            "gamma": gamma[c * H_LOC : (c + 1) * H_LOC],
        }
        for c in range(N_CORES)
    ]
    return bass_utils.run_bass_kernel_spmd(
        _NC, in_maps, core_ids=list(range(N_CORES)), **spmd_kwargs
    )


def kernel(alpha, beta, gamma, seq_len):
    assert int(seq_len) == S, f"kernel hardcodes seq_len={S}, got {seq_len}"
    res = _run(alpha, beta, gamma)
    return np.concatenate([r["out"] for r in res.results], axis=0)
